# revision 2
# baseline (speedup 1.0000x reference)
"""MiniAttentionQHead Trainium2 kernel v4 (8-core data parallel, all-bf16).

Single pool scope per iteration; no PE transposes / no mode switches;
V-fold rides the K-stream's stationary (u matmul per c-chunk, natural
[row, ch] output); K PSUM split into two [128,1024] halves (3 bufs, 6
banks) + vpp (1 bank) + q quarter (1 bank) = exactly 8 banks.
Column count is the exact-math minimum: 1.196M per core.
"""

import math

import numpy as np
import ml_dtypes

B, H, NH, W, A = 4096, 2048, 16, 8, 2
D = H // NH  # 128
NCORES = 8
R = B // NCORES  # 512 rows per core
NT = R // 128  # 4 row tiles
KC = H // 128  # 16 contraction chunks
NTOK = W  # 8 distinct kv tokens (hidden + 7 ctx)

_cache = {}

def _patch_tile_framework():
    """This environment's walrus accepts only ONE semaphore wait per
    instruction; Tile attaches several.  Patch the end-of-kernel drain and
    add a post-pass that hoists excess waits onto preceding same-engine
    NOPs (engine queues execute sequentially, so semantics are identical).
    """
    import concourse.tile as tile
    from concourse import mybir
    from concourse.vector_clock import ScopedClock

    if getattr(tile.TileContext, "_ant_drain_patched", False):
        return

    def patched(self, tick_clock, wait_clock):
        drain_inst = self.nc.sync.drain()
        wait_clock.add_sem_waits(
            drain_inst.ins, ScopedClock({None: tick_clock.global_clock})
        )
        si = drain_inst.ins.sync_info
        waits = list(si.on_wait or [])
        if len(waits) > 1:
            si.on_wait = waits[:1]
            for w in waits[1:]:
                extra = self.nc.sync.drain()
                extra.ins.sync_info = mybir.SyncInfo(on_wait=[w], on_update=[])
        self.nc.all_engine_barrier()
        assert self.sems is not None
        popped = self.nc._tile_sem_poison_stack.pop()
        assert popped is self._sem_poison
        self.nc.clear_and_free_semaphores(list(self.sems.allocated().values()))
        self.nc.all_engine_barrier()

    tile.TileContext._drain_and_barrier = patched
    tile.TileContext._ant_drain_patched = True


def _split_waits(nc, max_waits=1):
    from concourse import mybir

    cnt = 0
    for fn in nc.m.functions:
        for bb in fn.blocks:
            changed = False
            out = []
            for inst in bb.instructions:
                si = inst.sync_info
                if si is not None:
                    waits = list(si.on_wait or [])
                    if len(waits) > max_waits:
                        extra = waits[:-max_waits]
                        for k in range(0, len(extra), max_waits):
                            nop = mybir.InstNoOp(
                                name=f"I-antws-{cnt}", ins=[], outs=[]
                            )
                            cnt += 1
                            nop.engine = inst.engine
                            nop.sync_info = mybir.SyncInfo(
                                on_wait=extra[k : k + max_waits], on_update=[]
                            )
                            out.append(nop)
                        inst.sync_info = mybir.SyncInfo(
                            on_wait=waits[-max_waits:],
                            on_update=list(si.on_update or []),
                        )
                        changed = True
                out.append(inst)
            if changed:
                bb.instructions = out




def _build_nc(reps=1):
    key = ("nc4", reps)
    if key in _cache:
        return _cache[key]

    import concourse.bass as bass
    import concourse.tile as tile
    from concourse import mybir

    _patch_tile_framework()

    f32 = mybir.dt.float32
    bf16 = mybir.dt.bfloat16
    X = mybir.AxisListType.X
    XY = mybir.AxisListType.XY
    ADD = mybir.AluOpType.add
    MAX = mybir.AluOpType.max
    COPY = mybir.ActivationFunctionType.Copy

    nc = bass.Bass(target_bir_lowering=False)

    hid_d = nc.dram_tensor("hidK", [128, KC, R], bf16, kind="ExternalInput")
    ctx_d = nc.dram_tensor(
        "ctxK", [W - 1, NT, 128, KC, 128], bf16, kind="ExternalInput"
    )
    wq_d = nc.dram_tensor(
        "wqQ", [4, 128, KC, 512], bf16, kind="ExternalInput"
    )
    wk_d = nc.dram_tensor("wkK", [128, KC, H], bf16, kind="ExternalInput")
    u_d = nc.dram_tensor("uK", [128, KC, 2 * NH], bf16, kind="ExternalInput")
    out_d = nc.dram_tensor("qout", [R, A], f32, kind="ExternalOutput")

    qscale = 1.0 / math.sqrt(D)

    with tile.TileContext(nc) as tc:
        with tc.tile_pool(name="outer", bufs=1) as outer:
            hid_sb = outer.tile([128, KC, R], bf16, tag="hid")
            for q4 in range(4):
                nc.sync.dma_start(
                    out=hid_sb[:, 4 * q4 : 4 * q4 + 4, :],
                    in_=hid_d[:, 4 * q4 : 4 * q4 + 4, :],
                )
            wk_sb = outer.tile([128, KC, H], bf16, tag="wk")
            for c4 in range(4):
                nc.sync.dma_start(
                    out=wk_sb[:, 4 * c4 : 4 * c4 + 4, :],
                    in_=wk_d[:, 4 * c4 : 4 * c4 + 4, :],
                )
            u_sb = outer.tile([128, KC, 2 * NH], bf16, tag="u")
            nc.sync.dma_start(out=u_sb, in_=u_d[:, :, :])

            out_sbs = [
                outer.tile([128, A], f32, tag=f"out{t}", name=f"out{t}")
                for t in range(NT)
            ]
            q_sbs = [
                outer.tile([128, H], bf16, tag=f"q{t}", name=f"q{t}")
                for t in range(NT)
            ]
            sc_sbs = [
                outer.tile([128, NTOK, NH], bf16, tag=f"sc{t}", name=f"sc{t}")
                for t in range(NT)
            ]
            vp_sbs = [
                outer.tile(
                    [128, NTOK, 2 * NH], f32, tag=f"vp{t}", name=f"vp{t}"
                )
                for t in range(NT)
            ]

            for _rep in range(reps):
                with (
                    tc.tile_pool(name="wqs", bufs=2) as wqs,
                    tc.tile_pool(name="ctx", bufs=4) as ctxp,
                    tc.tile_pool(name="kbb", bufs=2) as kbbp,
                    tc.tile_pool(name="prod", bufs=2) as prodp,
                    tc.tile_pool(name="sm", bufs=2) as smp,
                    tc.tile_pool(name="qps", bufs=1, space="PSUM") as qps,
                    tc.tile_pool(name="kbh", bufs=3, space="PSUM") as kbhp,
                    tc.tile_pool(name="vps", bufs=1, space="PSUM") as vps,
                ):
                    # ---- Q-proj: quarters, one PSUM bank
                    for qt in range(4):
                        wq_sb = wqs.tile([128, KC, 512], bf16, tag="wq")
                        nc.sync.dma_start(out=wq_sb, in_=wq_d[qt])
                        for t in range(NT):
                            q_ps = qps.tile(
                                [128, 512], f32, tag="qp", name="qp"
                            )
                            for c in range(KC):
                                nc.tensor.matmul(
                                    q_ps,
                                    hid_sb[:, c, t * 128 : (t + 1) * 128],
                                    wq_sb[:, c, :],
                                    start=(c == 0),
                                    stop=(c == KC - 1),
                                )
                            nc.scalar.activation(
                                out=q_sbs[t][:, qt * 512 : (qt + 1) * 512],
                                in_=q_ps,
                                func=COPY,
                                scale=qscale,
                            )

                    # ---- K + V: per (t, j): wk halves + u ride one c-loop
                    for t in range(NT):
                        for j in range(NTOK):
                            if j == 0:
                                tok = None
                            else:
                                tok = ctxp.tile([128, KC, 128], bf16, tag="ct")
                                nc.sync.dma_start(
                                    out=tok, in_=ctx_d[j - 1, t]
                                )
                            ka = kbhp.tile([128, 1024], f32, tag="kb", name="ka")
                            kb = kbhp.tile([128, 1024], f32, tag="kb", name="kb")
                            vpp = vps.tile([128, 2 * NH], f32, tag="vp")
                            for c in range(KC):
                                lhs = (
                                    hid_sb[:, c, t * 128 : (t + 1) * 128]
                                    if j == 0
                                    else tok[:, c, :]
                                )
                                nc.tensor.matmul(
                                    ka[:, 0:512], lhs, wk_sb[:, c, 0:512],
                                    start=(c == 0), stop=(c == KC - 1),
                                )
                                nc.tensor.matmul(
                                    ka[:, 512:1024], lhs,
                                    wk_sb[:, c, 512:1024],
                                    start=(c == 0), stop=(c == KC - 1),
                                )
                                nc.tensor.matmul(
                                    kb[:, 0:512], lhs, wk_sb[:, c, 1024:1536],
                                    start=(c == 0), stop=(c == KC - 1),
                                )
                                nc.tensor.matmul(
                                    kb[:, 512:1024], lhs,
                                    wk_sb[:, c, 1536:2048],
                                    start=(c == 0), stop=(c == KC - 1),
                                )
                                nc.tensor.matmul(
                                    vpp, lhs, u_sb[:, c, :],
                                    start=(c == 0), stop=(c == KC - 1),
                                )
                            kbb = kbbp.tile([128, H], bf16, tag="kbb")
                            nc.scalar.activation(
                                out=kbb[:, 0:1024], in_=ka, func=COPY
                            )
                            nc.scalar.activation(
                                out=kbb[:, 1024:2048], in_=kb, func=COPY
                            )
                            nc.scalar.activation(
                                out=vp_sbs[t][:, j, :], in_=vpp, func=COPY
                            )
                            pr = prodp.tile([128, H], bf16, tag="pr")
                            nc.vector.tensor_mul(pr, kbb, q_sbs[t])
                            with nc.allow_low_precision(
                                reason="scores tolerate bf16 (2e-2 gate)"
                            ):
                                nc.vector.tensor_reduce(
                                    out=sc_sbs[t][:, j, :],
                                    in_=pr.rearrange("p (h d) -> p h d", d=D),
                                    axis=X,
                                    op=ADD,
                                )

                        # softmax + combine for tile t
                        scv = sc_sbs[t].rearrange("p j h -> p h j")
                        mx = smp.tile([128, NH], f32, tag=f"m{t}", name=f"m{t}")
                        nc.vector.tensor_reduce(
                            out=mx, in_=scv, axis=X, op=MAX
                        )
                        et = smp.tile(
                            [128, NH, NTOK], f32, tag=f"e{t}", name=f"e{t}"
                        )
                        for j in range(NTOK):
                            nc.vector.tensor_sub(
                                et[:, :, j], sc_sbs[t][:, j, :], mx
                            )
                        nc.scalar.activation(
                            out=et, in_=et,
                            func=mybir.ActivationFunctionType.Exp,
                        )
                        s8 = smp.tile([128, NH], f32, tag=f"s{t}", name=f"s{t}")
                        nc.vector.tensor_reduce(out=s8, in_=et, axis=X, op=ADD)
                        nc.vector.tensor_add(s8, s8, et[:, :, 0])
                        rcp = smp.tile(
                            [128, NH], f32, tag=f"r{t}", name=f"r{t}"
                        )
                        nc.vector.reciprocal(rcp, s8)
                        at = smp.tile(
                            [128, NH, NTOK], f32, tag=f"a{t}", name=f"a{t}"
                        )
                        for j in range(NTOK):
                            nc.vector.tensor_mul(at[:, :, j], et[:, :, j], rcp)
                        vv = vp_sbs[t].rearrange("p j (h a) -> p h j a", a=A)
                        for a in range(A):
                            tmp = smp.tile(
                                [128, NH, NTOK], f32,
                                tag=f"tm{t}", name=f"tm{t}",
                            )
                            nc.vector.tensor_mul(tmp, at, vv[:, :, :, a])
                            r1 = smp.tile(
                                [128, 1], f32, tag=f"r1{t}", name=f"r1{t}"
                            )
                            r2 = smp.tile(
                                [128, 1], f32, tag=f"r2{t}", name=f"r2{t}"
                            )
                            nc.vector.tensor_reduce(
                                out=r1, in_=tmp, axis=XY, op=ADD
                            )
                            nc.vector.tensor_reduce(
                                out=r2, in_=tmp[:, :, 0], axis=X, op=ADD
                            )
                            nc.vector.tensor_add(r1, r1, r2)
                            nc.vector.tensor_copy(
                                out=out_sbs[t][:, a : a + 1], in_=r1
                            )

            for t in range(NT):
                nc.sync.dma_start(
                    out=out_d[t * 128 : (t + 1) * 128, :], in_=out_sbs[t]
                )

    _split_waits(nc)
    _cache[key] = nc
    return nc


def _prep_inputs(hidden_state, context_buffer, w_qkv, w_out, b_out, context_ptr):
    """Host-side sharding + [p, ...]-major layouts + weight fold + bf16."""
    bf = ml_dtypes.bfloat16
    hidden_state = np.ascontiguousarray(hidden_state, dtype=np.float32)
    context_buffer = np.ascontiguousarray(context_buffer, dtype=np.float32)
    w_qkv = np.ascontiguousarray(w_qkv, dtype=np.float32)
    w_out = np.ascontiguousarray(w_out, dtype=np.float32)

    ptr = int(context_ptr) % W
    kept = [w for w in range(W) if w != ptr]

    wqT = w_qkv[0:H, :].T.reshape(KC, 128, H)  # [c, p, n]
    wqQ = np.ascontiguousarray(
        wqT.transpose(1, 0, 2).reshape(128, KC, 4, 512).transpose(2, 0, 1, 3)
    ).astype(bf)
    wkK = np.ascontiguousarray(
        w_qkv[H : 2 * H, :].T.reshape(KC, 128, H).transpose(1, 0, 2)
    ).astype(bf)
    wo = w_out.reshape(A, NH, D)
    wv = w_qkv[2 * H : 3 * H, :].reshape(NH, D, H)
    U = np.einsum("ahd,hdc->hac", wo, wv, optimize=True).reshape(2 * NH, H)
    uK = np.ascontiguousarray(
        U.T.reshape(KC, 128, 2 * NH).transpose(1, 0, 2)
    ).astype(bf)

    in_maps = []
    for core in range(NCORES):
        rows = slice(core * R, (core + 1) * R)
        hid = hidden_state[rows]  # [R, H]
        hidK = np.ascontiguousarray(
            hid.T.reshape(KC, 128, R).transpose(1, 0, 2)
        ).astype(bf)
        ctx = context_buffer[rows][:, kept, :]  # [R, 7, H]
        ctxK = np.ascontiguousarray(
            ctx.reshape(NT, 128, W - 1, KC, 128).transpose(2, 0, 4, 3, 1)
        ).astype(bf)
        in_maps.append(
            dict(hidK=hidK, ctxK=ctxK, wqQ=wqQ, wkK=wkK, uK=uK)
        )
    return in_maps


def kernel(hidden_state, context_buffer, w_qkv, w_out, b_out, context_ptr):
    from concourse.bass_utils import run_bass_kernel_spmd

    nc = _build_nc()
    in_maps = _prep_inputs(
        hidden_state, context_buffer, w_qkv, w_out, b_out, context_ptr
    )
    res = run_bass_kernel_spmd(nc, in_maps, core_ids=list(range(NCORES)))
    out = np.concatenate([r["qout"] for r in res.results], axis=0)
    return (out + np.asarray(b_out, dtype=np.float32)[None, :]).astype(
        np.float32
    )


# revision 4
# speedup vs baseline: 1.0669x; 1.0669x over previous
"""MiniAttentionQHead Trainium2 kernel v4 (8-core data parallel, all-bf16).

Single pool scope per iteration; no PE transposes / no mode switches;
V-fold rides the K-stream's stationary (u matmul per c-chunk, natural
[row, ch] output); K PSUM split into two [128,1024] halves (3 bufs, 6
banks) + vpp (1 bank) + q quarter (1 bank) = exactly 8 banks.
Column count is the exact-math minimum: 1.196M per core.
"""

import math

import numpy as np
import ml_dtypes

B, H, NH, W, A = 4096, 2048, 16, 8, 2
D = H // NH  # 128
NCORES = 8
R = B // NCORES  # 512 rows per core
NT = R // 128  # 4 row tiles
KC = H // 128  # 16 contraction chunks
NTOK = W  # 8 distinct kv tokens (hidden + 7 ctx)

_cache = {}

def _patch_tile_framework():
    """This environment's walrus accepts only ONE semaphore wait per
    instruction; Tile attaches several.  Patch the end-of-kernel drain and
    add a post-pass that hoists excess waits onto preceding same-engine
    NOPs (engine queues execute sequentially, so semantics are identical).
    """
    import concourse.tile as tile
    from concourse import mybir
    from concourse.vector_clock import ScopedClock

    if getattr(tile.TileContext, "_ant_drain_patched", False):
        return

    def patched(self, tick_clock, wait_clock):
        drain_inst = self.nc.sync.drain()
        wait_clock.add_sem_waits(
            drain_inst.ins, ScopedClock({None: tick_clock.global_clock})
        )
        si = drain_inst.ins.sync_info
        waits = list(si.on_wait or [])
        if len(waits) > 1:
            si.on_wait = waits[:1]
            for w in waits[1:]:
                extra = self.nc.sync.drain()
                extra.ins.sync_info = mybir.SyncInfo(on_wait=[w], on_update=[])
        self.nc.all_engine_barrier()
        assert self.sems is not None
        popped = self.nc._tile_sem_poison_stack.pop()
        assert popped is self._sem_poison
        self.nc.clear_and_free_semaphores(list(self.sems.allocated().values()))
        self.nc.all_engine_barrier()

    tile.TileContext._drain_and_barrier = patched
    tile.TileContext._ant_drain_patched = True


def _split_waits(nc, max_waits=1):
    from concourse import mybir

    cnt = 0
    for fn in nc.m.functions:
        for bb in fn.blocks:
            changed = False
            out = []
            for inst in bb.instructions:
                si = inst.sync_info
                if si is not None:
                    waits = list(si.on_wait or [])
                    if len(waits) > max_waits:
                        extra = waits[:-max_waits]
                        for k in range(0, len(extra), max_waits):
                            nop = mybir.InstNoOp(
                                name=f"I-antws-{cnt}", ins=[], outs=[]
                            )
                            cnt += 1
                            nop.engine = inst.engine
                            nop.sync_info = mybir.SyncInfo(
                                on_wait=extra[k : k + max_waits], on_update=[]
                            )
                            out.append(nop)
                        inst.sync_info = mybir.SyncInfo(
                            on_wait=waits[-max_waits:],
                            on_update=list(si.on_update or []),
                        )
                        changed = True
                out.append(inst)
            if changed:
                bb.instructions = out




def _build_nc(reps=1):
    key = ("nc6", reps)
    if key in _cache:
        return _cache[key]

    import concourse.bass as bass
    import concourse.tile as tile
    from concourse import mybir

    _patch_tile_framework()

    f32 = mybir.dt.float32
    bf16 = mybir.dt.bfloat16
    X = mybir.AxisListType.X
    XY = mybir.AxisListType.XY
    ADD = mybir.AluOpType.add
    MAX = mybir.AluOpType.max
    COPY = mybir.ActivationFunctionType.Copy

    nc = bass.Bass(target_bir_lowering=False)

    hid_d = nc.dram_tensor("hidK", [128, KC, R], bf16, kind="ExternalInput")
    ctx_d = nc.dram_tensor(
        "ctxK", [W - 1, NT, 128, KC, 128], bf16, kind="ExternalInput"
    )
    wq_d = nc.dram_tensor(
        "wqQ", [4, 128, KC, 512], bf16, kind="ExternalInput"
    )
    wk_d = nc.dram_tensor("wkK", [128, KC, H], bf16, kind="ExternalInput")
    u_d = nc.dram_tensor("uK", [128, KC, 2 * NH], bf16, kind="ExternalInput")
    out_d = nc.dram_tensor("qout", [R, A], f32, kind="ExternalOutput")

    qscale = 1.0 / math.sqrt(D)

    with tile.TileContext(nc) as tc:
        with tc.tile_pool(name="outer", bufs=1) as outer:
            hid_sb = outer.tile([128, KC, R], bf16, tag="hid")
            for q4 in range(4):
                nc.sync.dma_start(
                    out=hid_sb[:, 4 * q4 : 4 * q4 + 4, :],
                    in_=hid_d[:, 4 * q4 : 4 * q4 + 4, :],
                )
            wk_sb = outer.tile([128, KC, H], bf16, tag="wk")
            for c4 in range(4):
                nc.sync.dma_start(
                    out=wk_sb[:, 4 * c4 : 4 * c4 + 4, :],
                    in_=wk_d[:, 4 * c4 : 4 * c4 + 4, :],
                )
            u_sb = outer.tile([128, KC, 2 * NH], bf16, tag="u")
            nc.sync.dma_start(out=u_sb, in_=u_d[:, :, :])

            out_sbs = [
                outer.tile([128, A], f32, tag=f"out{t}", name=f"out{t}")
                for t in range(NT)
            ]
            q_sbs = [
                outer.tile([128, H], bf16, tag=f"q{t}", name=f"q{t}")
                for t in range(NT)
            ]
            sc_sbs = [
                outer.tile([128, NTOK, NH], bf16, tag=f"sc{t}", name=f"sc{t}")
                for t in range(NT)
            ]
            vp_sbs = [
                outer.tile(
                    [128, NTOK, 2 * NH], f32, tag=f"vp{t}", name=f"vp{t}"
                )
                for t in range(NT)
            ]

            for _rep in range(reps):
                with (
                    tc.tile_pool(name="wqs", bufs=2) as wqs,
                    tc.tile_pool(name="ctx", bufs=6) as ctxp,
                    tc.tile_pool(name="prod", bufs=3) as prodp,
                    tc.tile_pool(name="sm", bufs=3) as smp,
                    tc.tile_pool(name="qps", bufs=1, space="PSUM") as qps,
                    tc.tile_pool(name="kbh", bufs=3, space="PSUM") as kbhp,
                    tc.tile_pool(name="vps", bufs=1, space="PSUM") as vps,
                ):
                    # ---- Q-proj: quarters, one PSUM bank
                    for qt in range(4):
                        wq_sb = wqs.tile([128, KC, 512], bf16, tag="wq")
                        nc.sync.dma_start(out=wq_sb, in_=wq_d[qt])
                        for t in range(NT):
                            q_ps = qps.tile(
                                [128, 512], f32, tag="qp", name="qp"
                            )
                            for c in range(KC):
                                nc.tensor.matmul(
                                    q_ps,
                                    hid_sb[:, c, t * 128 : (t + 1) * 128],
                                    wq_sb[:, c, :],
                                    start=(c == 0),
                                    stop=(c == KC - 1),
                                )
                            nc.scalar.activation(
                                out=q_sbs[t][:, qt * 512 : (qt + 1) * 512],
                                in_=q_ps,
                                func=COPY,
                                scale=qscale,
                            )

                    # ---- K + V: per (t, j): wk halves + u ride one c-loop
                    for t in range(NT):
                        for j in range(NTOK):
                            if j == 0:
                                tok = None
                            else:
                                tok = ctxp.tile([128, KC, 128], bf16, tag="ct")
                                nc.sync.dma_start(
                                    out=tok, in_=ctx_d[j - 1, t]
                                )
                            ka = kbhp.tile([128, 1024], f32, tag="kb", name="ka")
                            kb = kbhp.tile([128, 1024], f32, tag="kb", name="kb")
                            vpp = vps.tile([128, 2 * NH], f32, tag="vp")
                            for c in range(KC):
                                lhs = (
                                    hid_sb[:, c, t * 128 : (t + 1) * 128]
                                    if j == 0
                                    else tok[:, c, :]
                                )
                                nc.tensor.matmul(
                                    ka[:, 0:512], lhs, wk_sb[:, c, 0:512],
                                    start=(c == 0), stop=(c == KC - 1),
                                )
                                nc.tensor.matmul(
                                    ka[:, 512:1024], lhs,
                                    wk_sb[:, c, 512:1024],
                                    start=(c == 0), stop=(c == KC - 1),
                                )
                                nc.tensor.matmul(
                                    kb[:, 0:512], lhs, wk_sb[:, c, 1024:1536],
                                    start=(c == 0), stop=(c == KC - 1),
                                )
                                nc.tensor.matmul(
                                    kb[:, 512:1024], lhs,
                                    wk_sb[:, c, 1536:2048],
                                    start=(c == 0), stop=(c == KC - 1),
                                )
                                nc.tensor.matmul(
                                    vpp, lhs, u_sb[:, c, :],
                                    start=(c == 0), stop=(c == KC - 1),
                                )
                            nc.scalar.activation(
                                out=vp_sbs[t][:, j, :], in_=vpp, func=COPY
                            )
                            pr = prodp.tile([128, H], bf16, tag="pr")
                            nc.vector.tensor_mul(
                                pr[:, 0:1024], ka, q_sbs[t][:, 0:1024]
                            )
                            nc.vector.tensor_mul(
                                pr[:, 1024:2048], kb, q_sbs[t][:, 1024:2048]
                            )
                            with nc.allow_low_precision(
                                reason="scores tolerate bf16 (2e-2 gate)"
                            ):
                                nc.vector.tensor_reduce(
                                    out=sc_sbs[t][:, j, :],
                                    in_=pr.rearrange("p (h d) -> p h d", d=D),
                                    axis=X,
                                    op=ADD,
                                )

                        # softmax + combine for tile t
                        scv = sc_sbs[t].rearrange("p j h -> p h j")
                        mx = smp.tile([128, NH], f32, tag=f"m{t}", name=f"m{t}")
                        nc.vector.tensor_reduce(
                            out=mx, in_=scv, axis=X, op=MAX
                        )
                        et = smp.tile(
                            [128, NH, NTOK], f32, tag=f"e{t}", name=f"e{t}"
                        )
                        for j in range(NTOK):
                            nc.vector.tensor_sub(
                                et[:, :, j], sc_sbs[t][:, j, :], mx
                            )
                        nc.scalar.activation(
                            out=et, in_=et,
                            func=mybir.ActivationFunctionType.Exp,
                        )
                        s8 = smp.tile([128, NH], f32, tag=f"s{t}", name=f"s{t}")
                        nc.vector.tensor_reduce(out=s8, in_=et, axis=X, op=ADD)
                        nc.vector.tensor_add(s8, s8, et[:, :, 0])
                        rcp = smp.tile(
                            [128, NH], f32, tag=f"r{t}", name=f"r{t}"
                        )
                        nc.vector.reciprocal(rcp, s8)
                        at = smp.tile(
                            [128, NH, NTOK], f32, tag=f"a{t}", name=f"a{t}"
                        )
                        for j in range(NTOK):
                            nc.vector.tensor_mul(at[:, :, j], et[:, :, j], rcp)
                        vv = vp_sbs[t].rearrange("p j (h a) -> p h j a", a=A)
                        for a in range(A):
                            tmp = smp.tile(
                                [128, NH, NTOK], f32,
                                tag=f"tm{t}", name=f"tm{t}",
                            )
                            nc.vector.tensor_mul(tmp, at, vv[:, :, :, a])
                            r1 = smp.tile(
                                [128, 1], f32, tag=f"r1{t}", name=f"r1{t}"
                            )
                            r2 = smp.tile(
                                [128, 1], f32, tag=f"r2{t}", name=f"r2{t}"
                            )
                            nc.vector.tensor_reduce(
                                out=r1, in_=tmp, axis=XY, op=ADD
                            )
                            nc.vector.tensor_reduce(
                                out=r2, in_=tmp[:, :, 0], axis=X, op=ADD
                            )
                            nc.vector.tensor_add(r1, r1, r2)
                            nc.vector.tensor_copy(
                                out=out_sbs[t][:, a : a + 1], in_=r1
                            )

            for t in range(NT):
                nc.sync.dma_start(
                    out=out_d[t * 128 : (t + 1) * 128, :], in_=out_sbs[t]
                )

    _split_waits(nc)
    _cache[key] = nc
    return nc


def _prep_inputs(hidden_state, context_buffer, w_qkv, w_out, b_out, context_ptr):
    """Host-side sharding + [p, ...]-major layouts + weight fold + bf16."""
    bf = ml_dtypes.bfloat16
    hidden_state = np.ascontiguousarray(hidden_state, dtype=np.float32)
    context_buffer = np.ascontiguousarray(context_buffer, dtype=np.float32)
    w_qkv = np.ascontiguousarray(w_qkv, dtype=np.float32)
    w_out = np.ascontiguousarray(w_out, dtype=np.float32)

    ptr = int(context_ptr) % W
    kept = [w for w in range(W) if w != ptr]

    wqT = w_qkv[0:H, :].T.reshape(KC, 128, H)  # [c, p, n]
    wqQ = np.ascontiguousarray(
        wqT.transpose(1, 0, 2).reshape(128, KC, 4, 512).transpose(2, 0, 1, 3)
    ).astype(bf)
    wkK = np.ascontiguousarray(
        w_qkv[H : 2 * H, :].T.reshape(KC, 128, H).transpose(1, 0, 2)
    ).astype(bf)
    wo = w_out.reshape(A, NH, D)
    wv = w_qkv[2 * H : 3 * H, :].reshape(NH, D, H)
    U = np.einsum("ahd,hdc->hac", wo, wv, optimize=True).reshape(2 * NH, H)
    uK = np.ascontiguousarray(
        U.T.reshape(KC, 128, 2 * NH).transpose(1, 0, 2)
    ).astype(bf)

    in_maps = []
    for core in range(NCORES):
        rows = slice(core * R, (core + 1) * R)
        hid = hidden_state[rows]  # [R, H]
        hidK = np.ascontiguousarray(
            hid.T.reshape(KC, 128, R).transpose(1, 0, 2)
        ).astype(bf)
        ctx = context_buffer[rows][:, kept, :]  # [R, 7, H]
        ctxK = np.ascontiguousarray(
            ctx.reshape(NT, 128, W - 1, KC, 128).transpose(2, 0, 4, 3, 1)
        ).astype(bf)
        in_maps.append(
            dict(hidK=hidK, ctxK=ctxK, wqQ=wqQ, wkK=wkK, uK=uK)
        )
    return in_maps


def kernel(hidden_state, context_buffer, w_qkv, w_out, b_out, context_ptr):
    from concourse.bass_utils import run_bass_kernel_spmd

    nc = _build_nc()
    in_maps = _prep_inputs(
        hidden_state, context_buffer, w_qkv, w_out, b_out, context_ptr
    )
    res = run_bass_kernel_spmd(nc, in_maps, core_ids=list(range(NCORES)))
    out = np.concatenate([r["qout"] for r in res.results], axis=0)
    return (out + np.asarray(b_out, dtype=np.float32)[None, :]).astype(
        np.float32
    )
